# revision 24
# baseline (speedup 1.0000x reference)
"""Bass/Trainium2 kernel for nn_BiHgru2_1d (bidirectional HGRU block), 8-core SPMD.

Math (reference):
    feat = x @ W_in.T + b_in                    # (N,B,3D)
    inp, og, fg = split(feat); inp=silu(inp); og=sigmoid(og); lam=sigmoid(fg)
    u[h,d,e] = (1-lam[h,d]) * inp[h,e];  lam_f[h,d,e] = lam[h,d]
    s = fwd_scan(lam_f, u) + rev_scan(lam_f, u)         # h_t = lam_t h_{t-1} + u_t
    o[h,e] = sum_d s[h,d,e]*og[h,d]; o = LN(o)*gamma+beta; out = o @ W_out.T + b_out

Sharding: 8-way tensor parallel over heads (128 heads/core). Each core:
  GEMM1 (x full, W_in 768-row slice, fp16) -> activations -> per-(b,d,e)
  tensor_tensor_scan fwd+rev (rev via negative-stride APs) -> o_acc ->
  AllToAll (reshard channel->token) -> LayerNorm -> GEMM2 (W_out full, fp16)
  -> writes its 1024-token slice of the output.

Sign trick: we compute u' = (lam-1)*inp = -u (saves an op; no rsub on HW),
so s' = -s and o' = -o; host passes gamma' = -gamma which makes
LN_{gamma',beta}(o') == LN_{gamma,beta}(o) exactly (mean/var sign-symmetric).
"""

import sys

for _p in ("/opt/trn_rl_repo",):
    if _p not in sys.path:
        sys.path.insert(0, _p)

import numpy as np

# ---- problem constants (hardcoded per contract) ----
N_FULL, B, D = 2048, 4, 2048
E = 2
H = D // E                      # 1024 heads
NCORES = 8
P = 128                         # partitions
HC = H // NCORES                # 128 heads per core
KC = D // P                     # 16 k-chunks
M_TILES = 6                     # [inp e0, inp e1, og e0, og e1, fg d0, fg d1]

_BUILD_CACHE = {}


def build_program(T=N_FULL, num_devices=NCORES, use_silu=True, debug_dump=False):
    """Build the SPMD Bass program (same program on every core)."""
    import concourse.bass as bass
    import concourse.mybir as mybir
    import concourse.tile as tile
    from concourse import bacc

    f16 = mybir.dt.float16
    f32 = mybir.dt.float32
    MUL = mybir.AluOpType.mult
    ADD = mybir.AluOpType.add
    SUB = mybir.AluOpType.subtract
    AF = mybir.ActivationFunctionType

    TOK_C = B * T // NCORES      # tokens per core after reshard (== T//2)
    NBLK = min(512, T)           # GEMM1 token-block size (per batch)
    NB1 = T // NBLK              # token blocks per batch
    TCH = min(P, TOK_C)          # GEMM2 token-chunk (output partition dim)
    NTCH = TOK_C // TCH
    OCB = 512                    # GEMM2 out-col block
    NOC = D // OCB
    NHALF = min(512, TOK_C)      # LN-stat matmul free-dim block
    assert B * T % NCORES == 0 and T % NBLK == 0 and TOK_C % TCH == 0

    nc = bacc.Bacc("TRN2", target_bir_lowering=False, debug=False,
                   num_devices=num_devices)

    # ---- per-core DRAM parameters ----
    xT_d = nc.dram_tensor("xT", [D, B * T], f16, kind="ExternalInput")
    w1T_d = nc.dram_tensor("w1T", [D, M_TILES * P], f16, kind="ExternalInput")
    b1_d = nc.dram_tensor("b1", [P, M_TILES], f32, kind="ExternalInput")
    w2T_d = nc.dram_tensor("w2T", [D, D], f16, kind="ExternalInput")
    gam_d = nc.dram_tensor("gam", [P, KC], f32, kind="ExternalInput")
    c1_d = nc.dram_tensor("c1r", [P, D], f32, kind="ExternalInput")   # gamma@W2T
    c2_d = nc.dram_tensor("c2r", [P, D], f32, kind="ExternalInput")   # beta@W2T+b_out
    out_d = nc.dram_tensor("out", [TOK_C, D], f32, kind="ExternalOutput")
    if debug_dump:
        dbg_lam = nc.dram_tensor("dbg_lam", [P, B, E, T], f16, kind="ExternalOutput")
        dbg_inp = nc.dram_tensor("dbg_inp", [P, B, E, T], f16, kind="ExternalOutput")
        dbg_og = nc.dram_tensor("dbg_og", [P, B, E, T], f16, kind="ExternalOutput")
        dbg_oac = nc.dram_tensor("dbg_oac", [P, B, E, T], f16, kind="ExternalOutput")
        dbg_ot = nc.dram_tensor("dbg_ot", [P, KC, B * T // NCORES], f16,
                                kind="ExternalOutput")
        dbg_ab = nc.dram_tensor("dbg_ab", [TCH, 2 * NTCH], f32,
                                kind="ExternalOutput")

    xT_r = xT_d.ap().rearrange("(kc p) t -> p kc t", p=P)
    w1T_r = w1T_d.ap().rearrange("(kc p) m -> p kc m", p=P)
    w2T_r = w2T_d.ap().rearrange("(kc p) o -> p kc o", p=P)

    with tile.TileContext(nc) as tc:
        with (
            tc.tile_pool(name="cst", bufs=1) as cst_pool,
            tc.tile_pool(name="ps", bufs=8, space="PSUM") as psum_pool,
            tc.tile_pool(name="dram", bufs=1, space="DRAM") as dram_pool,
        ):
            # internal DRAM bounce buffers for the AllToAll
            cc_in = dram_pool.tile([NCORES, P, E, TOK_C], f16, tag="cc_in")
            cc_out = dram_pool.tile([NCORES, P, E, TOK_C], f16, tag="cc_out")
            b1_sb = cst_pool.tile([P, M_TILES], f32, tag="b1")
            nc.sync.dma_start(b1_sb[:], b1_d.ap())
            gam_sb = cst_pool.tile([P, KC], f32, tag="gam")
            nc.sync.dma_start(gam_sb[:], gam_d.ap())
            ones_sb = cst_pool.tile([P, 1], f16, tag="ones")
            nc.vector.memset(ones_sb[:], 1.0)
            eps_sb = cst_pool.tile([1, 1], f32, tag="eps")
            nc.vector.memset(eps_sb[:], 1e-5)

            # ================= Phase A: GEMM1 + scans =================
            with (
                tc.tile_pool(name="w1p", bufs=1) as w1_pool,
                tc.tile_pool(name="res", bufs=1) as res_pool,
                tc.tile_pool(name="xs", bufs=2) as x_pool,
                tc.tile_pool(name="scan", bufs=1) as scan_pool,
            ):
                w1_sb = w1_pool.tile([P, KC, M_TILES * P], f16, tag="w1")
                for q in range(4):
                    nc.sync.dma_start(w1_sb[:, 4 * q:4 * (q + 1), :],
                                      w1T_r[:, 4 * q:4 * (q + 1), :])

                # residents: [128 heads; b, e-or-d, t] fp16
                lam_res = res_pool.tile([P, B, E, T], f16, tag="lam")
                inp_res = res_pool.tile([P, B, E, T], f16, tag="inp")
                og_res = res_pool.tile([P, B, E, T], f16, tag="og")
                oac_res = res_pool.tile([P, B, E, T], f16, tag="oac")

                # epilogue routing: m-tile -> (dest, activation)
                dests = [(inp_res, AF.Silu), (inp_res, AF.Silu),
                         (og_res, AF.Sigmoid), (og_res, AF.Sigmoid),
                         (lam_res, AF.Sigmoid), (lam_res, AF.Sigmoid)]

                for b in range(B):
                    # ---- GEMM1 for this batch's tokens ----
                    for nb in range(NB1):
                        tok0 = b * T + nb * NBLK
                        xt = x_pool.tile([P, KC, NBLK], f16, tag="xt")
                        for q in range(4):
                            nc.sync.dma_start(
                                xt[:, 4 * q:4 * (q + 1), :],
                                xT_r[:, 4 * q:4 * (q + 1), tok0:tok0 + NBLK])
                        for m in range(M_TILES):
                            ps = psum_pool.tile([P, NBLK], f32, tag="ps")
                            for kc in range(KC):
                                nc.tensor.matmul(
                                    ps[:], w1_sb[:, kc, m * P:(m + 1) * P],
                                    xt[:, kc, :],
                                    start=(kc == 0), stop=(kc == KC - 1))
                            dest, func = dests[m]
                            dsl = dest[:, b, m % 2, nb * NBLK:(nb + 1) * NBLK]
                            if func == AF.Silu and not use_silu:
                                # sim fallback: silu(x) = x * sigmoid(x)
                                sg = x_pool.tile([P, NBLK], f32, tag="sg")
                                nc.scalar.activation(sg[:], ps[:], AF.Sigmoid,
                                                     bias=b1_sb[:, m:m + 1])
                                nc.scalar.activation(ps[:], ps[:], AF.Identity,
                                                     bias=b1_sb[:, m:m + 1])
                                nc.vector.tensor_tensor(dsl, ps[:], sg[:], MUL)
                            else:
                                nc.scalar.activation(dsl, ps[:], func,
                                                     bias=b1_sb[:, m:m + 1])

                    # ---- scans for this batch ----
                    for rev in (False, True):
                        for d in range(E):
                            lam_bd = lam_res[:, b, d, :]
                            og_bd = og_res[:, b, d, :]
                            for e in range(E):
                                u = scan_pool.tile([P, T], f16, tag="u")
                                # u' = (lam-1)*inp = -u
                                nc.vector.scalar_tensor_tensor(
                                    u[:], lam_bd, 1.0, inp_res[:, b, e, :],
                                    op0=SUB, op1=MUL)
                                s = scan_pool.tile([P, T], f16, tag="s")
                                if rev:
                                    nc.vector.tensor_tensor_scan(
                                        s[:, ::-1], lam_bd[:, ::-1],
                                        u[:, ::-1], 0.0, op0=MUL, op1=ADD)
                                else:
                                    nc.vector.tensor_tensor_scan(
                                        s[:], lam_bd, u[:], 0.0,
                                        op0=MUL, op1=ADD)
                                o_be = oac_res[:, b, e, :]
                                if not rev and d == 0:
                                    nc.vector.tensor_tensor(o_be, og_bd, s[:], MUL)
                                else:
                                    tmp = scan_pool.tile([P, T], f16, tag="tmp")
                                    nc.vector.tensor_tensor(tmp[:], og_bd, s[:], MUL)
                                    nc.vector.tensor_tensor(o_be, o_be, tmp[:], ADD)

                if debug_dump:
                    nc.sync.dma_start(dbg_lam.ap(), lam_res[:])
                    nc.sync.dma_start(dbg_inp.ap(), inp_res[:])
                    nc.sync.dma_start(dbg_og.ap(), og_res[:])
                    nc.sync.dma_start(dbg_oac.ap(), oac_res[:])

                # ---- stage AllToAll input ----
                for j in range(NCORES):
                    bj, hj = j // 2, j % 2
                    nsl = slice(hj * TOK_C, (hj + 1) * TOK_C)
                    nc.gpsimd.dma_start(cc_in[j], oac_res[:, bj, :, nsl])

            nc.gpsimd.collective_compute(
                "AllToAll", mybir.AluOpType.bypass,
                replica_groups=[list(range(NCORES))],
                ins=[cc_in.opt()], outs=[cc_out.opt()])

            # ============ Phase B: LayerNorm + GEMM2 ============
            # LN folded into GEMM2 epilogue:
            #   out[t,:] = a_t * (gamma.o_t)@W2T + b_t * c1 + c2
            # with on-chip o' = -o, a_t = -rstd_t, b_t = -rstd_t * mu'_t,
            # c1 = gamma@W2T, c2 = beta@W2T + b_out (host constants).
            with (
                tc.tile_pool(name="otp", bufs=1) as ot_pool,
                tc.tile_pool(name="w2p", bufs=2) as w2_pool,
                tc.tile_pool(name="stat", bufs=1) as stat_pool,
                tc.tile_pool(name="sc2", bufs=3) as sc2_pool,
            ):
                c1_sb = stat_pool.tile([P, D], f32, tag="c1")
                nc.sync.dma_start(c1_sb[:], c1_d.ap())
                c2_sb = stat_pool.tile([P, D], f32, tag="c2")
                nc.sync.dma_start(c2_sb[:], c2_d.ap())

                cc_out_r = cc_out.rearrange("j p e t -> (j p e) t")
                ot_all = ot_pool.tile([P, KC, TOK_C], f16, tag="ot")
                for kc in range(KC):
                    nc.gpsimd.dma_start(ot_all[:, kc, :],
                                        cc_out_r[kc * P:(kc + 1) * P, :])

                # stats: sum(o'), sum(o'^2) over channels (partition dim) via PE
                sum_sb = stat_pool.tile([1, TOK_C], f32, tag="sum")
                ssq_sb = stat_pool.tile([1, TOK_C], f32, tag="ssq")
                for si, dst in ((0, sum_sb), (1, ssq_sb)):
                    for hf in range(TOK_C // NHALF):
                        fsl = slice(hf * NHALF, (hf + 1) * NHALF)
                        pss = psum_pool.tile([1, NHALF], f32, tag="ps")
                        for kc in range(KC):
                            if si == 0:
                                rhs = ot_all[:, kc, fsl]
                            else:
                                sq = sc2_pool.tile([P, NHALF], f16, tag="sq")
                                nc.vector.tensor_tensor(
                                    sq[:], ot_all[:, kc, fsl],
                                    ot_all[:, kc, fsl], MUL)
                                rhs = sq[:]
                            nc.tensor.matmul(pss[:], ones_sb[:], rhs,
                                             start=(kc == 0),
                                             stop=(kc == KC - 1))
                        nc.vector.tensor_copy(out=dst[:, fsl], in_=pss[:])

                mu = stat_pool.tile([1, TOK_C], f32, tag="mu")
                nc.scalar.mul(mu[:], sum_sb[:], 1.0 / D)
                var = stat_pool.tile([1, TOK_C], f32, tag="var")
                nc.vector.tensor_tensor(var[:], mu[:], mu[:], MUL)
                m2 = stat_pool.tile([1, TOK_C], f32, tag="m2")
                nc.scalar.mul(m2[:], ssq_sb[:], 1.0 / D)
                nc.vector.tensor_tensor(var[:], m2[:], var[:], SUB)
                std = stat_pool.tile([1, TOK_C], f32, tag="std")
                nc.scalar.activation(std[:], var[:], AF.Sqrt, bias=eps_sb[:])
                a_sb = stat_pool.tile([1, TOK_C], f32, tag="a")
                nc.vector.reciprocal(a_sb[:], std[:])          # rstd
                b_sb = stat_pool.tile([1, TOK_C], f32, tag="b")
                nc.vector.tensor_tensor(b_sb[:], a_sb[:], mu[:], MUL)  # rstd*mu'
                nc.scalar.mul(a_sb[:], a_sb[:], -1.0)          # a = -rstd

                # reshape a,b to per-partition [P, NTCH] via a DRAM bounce
                ab_dram = dram_pool.tile([2, TOK_C], f32, tag="ab")
                nc.sync.dma_start(ab_dram[0:1, :], a_sb[:])
                nc.sync.dma_start(ab_dram[1:2, :], b_sb[:])
                ab_r = ab_dram.rearrange("s (c p) -> s p c", p=TCH)
                aT_sb = stat_pool.tile([TCH, NTCH], f32, tag="aT")
                nc.sync.dma_start(aT_sb[:], ab_r[0])
                bT_sb = stat_pool.tile([TCH, NTCH], f32, tag="bT")
                nc.sync.dma_start(bT_sb[:], ab_r[1])

                if debug_dump:
                    nc.sync.dma_start(dbg_ot.ap(), ot_all[:])
                    nc.sync.dma_start(dbg_ab.ap()[:, :NTCH], aT_sb[:])
                    nc.sync.dma_start(dbg_ab.ap()[:, NTCH:], bT_sb[:])

                # scale by gamma in place (fp16): ot[kc] *= gamma[kc]
                for kc in range(KC):
                    nc.scalar.mul(ot_all[:, kc, :], ot_all[:, kc, :],
                                  gam_sb[:, kc:kc + 1])

                # GEMM2 + LN epilogue + output
                for oc in range(NOC):
                    w2 = w2_pool.tile([P, KC, OCB], f16, tag="w2")
                    ocs = slice(oc * OCB, (oc + 1) * OCB)
                    for q in range(4):
                        nc.sync.dma_start(w2[:, 4 * q:4 * (q + 1), :],
                                          w2T_r[:, 4 * q:4 * (q + 1), ocs])
                    for tch in range(NTCH):
                        ps2 = psum_pool.tile([TCH, OCB], f32, tag="ps")
                        for kc in range(KC):
                            nc.tensor.matmul(
                                ps2[:],
                                ot_all[:, kc, tch * TCH:(tch + 1) * TCH],
                                w2[:, kc, :],
                                start=(kc == 0), stop=(kc == KC - 1))
                        # tb = b_t * c1 + c2 ; out = a_t * G + tb
                        tb = sc2_pool.tile([TCH, OCB], f32, tag="tb")
                        nc.scalar.mul(tb[:], c1_sb[:TCH, ocs],
                                      bT_sb[:, tch:tch + 1])
                        nc.vector.tensor_tensor(tb[:], tb[:],
                                                c2_sb[:TCH, ocs], ADD)
                        ob = sc2_pool.tile([TCH, OCB], f32, tag="ob")
                        nc.vector.scalar_tensor_tensor(
                            ob[:], ps2[:], aT_sb[:, tch:tch + 1], tb[:],
                            op0=MUL, op1=ADD)
                        nc.sync.dma_start(
                            out_d.ap()[tch * TCH:(tch + 1) * TCH, ocs], ob[:])

    nc.compile()
    return nc


def host_prep(x, W_in, b_in, gamma, beta, W_out, b_out, T=N_FULL):
    """Host-side input prep: fp16 casts, transposes, per-core W_in slices."""
    x = np.asarray(x)
    gamma = np.asarray(gamma, np.float32)
    beta = np.asarray(beta, np.float32)
    W_out = np.asarray(W_out, np.float32)
    b_out = np.asarray(b_out, np.float32)
    xT = np.ascontiguousarray(np.asarray(x, np.float32).transpose(2, 1, 0)
                              .reshape(D, B * T)).astype(np.float16)
    w2T = np.ascontiguousarray(W_out.T).astype(np.float16)
    gam = np.ascontiguousarray(gamma.reshape(KC, P).T)
    c1 = gamma @ W_out.T
    c2 = beta @ W_out.T + b_out
    c1r = np.ascontiguousarray(np.broadcast_to(c1, (P, D)), dtype=np.float32)
    c2r = np.ascontiguousarray(np.broadcast_to(c2, (P, D)), dtype=np.float32)

    W_in = np.asarray(W_in, np.float32)
    b_in = np.asarray(b_in, np.float32)
    in_maps = []
    for c in range(NCORES):
        base = c * 2 * P
        rows = []
        for blk in range(3):                  # inp, og, fg
            for e in range(E):                # e0, e1 (or d0, d1 for fg)
                rows.append(blk * D + base + 2 * np.arange(P) + e)
        rows = np.concatenate(rows)           # (768,)
        w1T_c = np.ascontiguousarray(W_in[rows, :].T).astype(np.float16)
        b1_c = np.ascontiguousarray(b_in[rows].reshape(M_TILES, P).T)
        in_maps.append({
            "xT": xT, "w1T": w1T_c, "b1": b1_c, "w2T": w2T,
            "gam": gam, "c1r": c1r, "c2r": c2r,
        })
    return in_maps


def assemble_output(results, T=N_FULL):
    """Gather per-core [TOK_C, D] outputs into the full (N, B, D) array."""
    TOK_C = B * T // NCORES
    out = np.empty((T, B, D), np.float32)
    for i, res in enumerate(results):
        b, hj = i // 2, i % 2
        out[hj * TOK_C:(hj + 1) * TOK_C, b, :] = res["out"]
    return out


def kernel(x, W_in, b_in, gamma, beta, W_out, b_out):
    from concourse.bass_utils import run_bass_kernel_spmd

    key = N_FULL
    if key not in _BUILD_CACHE:
        _BUILD_CACHE[key] = build_program(N_FULL)
    nc = _BUILD_CACHE[key]
    in_maps = host_prep(x, W_in, b_in, gamma, beta, W_out, b_out)
    res = run_bass_kernel_spmd(nc, in_maps, core_ids=list(range(NCORES)))
    return assemble_output(res.results)


if __name__ == "__main__":
    import reference
    inputs = {k: np.asarray(v) for k, v in reference.setup_inputs().items()}
    expected = np.asarray(reference.reference(**inputs))
    actual = kernel(**inputs)
    err = np.abs(actual - expected)
    rel = np.linalg.norm(actual - expected) / np.linalg.norm(expected)
    print("max abs err:", err.max(), "rel fro err:", rel)
